# revision 7
# baseline (speedup 1.0000x reference)
"""PointPillarScatter on 8 NeuronCores.

Full inputs -> full (B, C, NX, NY) float32 output.

Sharding: core k handles (sample b = k//2, output-x half h = k%2); each core
produces out[b, :, h*216:(h+1)*216, :] (the flip along x is baked into the
host-built scatter offsets).

Per-core device pipeline, two phases:

  Phase 1 (sparse scatter, ~6k rows/core):
    The ~6k real pillar rows are DMA'd densely into SBUF and scattered by
    dma_scatter_add into a runtime-pre-zeroed DRAM staging canvas.  Staging is
    laid out partition-major: partition p owns 838 consecutive rows (837 canvas
    positions {i : i % 128 == p} ordered by i // 128, plus 1 dump row for the
    padding slots), so the offsets bake in both the scatter and the
    transpose-friendly permutation, and int16 offsets stay in range per
    32-partition region.

  Phase 2 (dense stream, memory-bound):
    Per chunk of 24 output-x rows: one big contiguous DMA pulls the staging
    slice into SBUF as [128 pos-in-block, 93 blocks, 64 ch]; PE transposes
    pairs of 128-position blocks through an identity ([128,128] -> PSUM);
    DVE/ACT copy PSUM into the [64 ch, 11904 pos] out tile; one 3 MB DMA
    writes the (C, X, Y) canvas slice.
"""

import sys

sys.path.insert(0, "/opt/trn_rl_repo")

import numpy as np

import concourse.bacc as bacc
import concourse.mybir as mybir
from concourse.bass_utils import run_bass_kernel_spmd
from concourse.masks import make_identity
from concourse.tile import TileContext

C = 64
NX = 432
NY = 496
B = 4
NCORES = 8
XH = NX // 2            # 216 x-rows per core
M = XH * NY             # 107136 positions per core
P = 128
JPP = M // P            # 837 real rows per partition
RPP = JPP + 1           # +1 dump row
NREG = 4                # int16 offset range => scatter per 32-partition region
PREG = P // NREG        # 32 partitions per region
REGROWS = PREG * RPP    # 26816 staging rows per region

XCHUNK = 8
NCHUNK = XH // XCHUNK   # 27
MC = XCHUNK * NY        # 3968 positions per chunk
JBLK = MC // P          # 31 blocks of 128 positions
NCHUNK_SPLIT = 14       # chunks 0..13 read j < 434; high-j scatter overlaps them
JSPLIT = NCHUNK_SPLIT * JBLK   # = 434

_CACHE = {}
LAST_RESULTS = None


def _build_program(jr):
    nslot = P * jr          # scatter slots per segment (padded, fixed count)
    NSEG = 2 * NREG         # (j-half, region)
    nc = bacc.Bacc(None, target_bir_lowering=False)
    feats = nc.dram_tensor("feats", [NSEG * nslot, C], mybir.dt.float32, kind="ExternalInput")
    sidx = nc.dram_tensor("sidx", [P, NSEG * nslot // 16], mybir.dt.int16, kind="ExternalInput")
    staging = nc.dram_tensor("staging", [P * RPP, C], mybir.dt.float32, kind="ExternalOutput")
    out = nc.dram_tensor("out", [C, XH, NY], mybir.dt.float32, kind="ExternalOutput")

    stview = staging[:].rearrange("(pt j) c -> pt j c", j=RPP)   # [128, 838, 64]

    def emit_scatter(scatp, sidxp, seg):
        ft = scatp.tile([P, jr, C], mybir.dt.float32, tag="ft")
        nc.scalar.dma_start(ft[:], feats[seg * nslot:(seg + 1) * nslot, :].rearrange("(p j) c -> p j c", j=jr))
        it = sidxp.tile([P, nslot // 16], mybir.dt.int16, tag="it")
        nc.scalar.dma_start(it[:], sidx[:, seg * (nslot // 16):(seg + 1) * (nslot // 16)])
        r = seg % NREG
        nc.gpsimd.dma_scatter_add(
            out_ap=staging[r * REGROWS:(r + 1) * REGROWS, :],
            in_ap=ft[:],
            idxs_ap=it[:],
            num_idxs=nslot,
            num_idxs_reg=nslot,
            elem_size=C,
            single_packet=False,
        )

    def emit_chunk(ci, gatherp, outp, psump, psumsp, ident):
        gt = gatherp.tile([P, JBLK * C], mybir.dt.float32, tag="gt")
        nc.scalar.dma_start(
            gt[:].rearrange("p (j c) -> p j c", c=C),
            stview[:, ci * JBLK:(ci + 1) * JBLK, :],
        )
        ot = outp.tile([C, MC], mybir.dt.float32, tag="ot")
        npairs = JBLK // 2
        nquads = (npairs + 3) // 4
        for q in range(nquads):
            np_q = min(4, npairs - q * 4)
            pt = psump.tile([P, 512], mybir.dt.float32, tag="pt")
            for m in range(np_q):
                k = q * 4 + m
                nc.tensor.transpose(pt[:, m * P:(m + 1) * P], gt[:, k * P:(k + 1) * P], ident[:])
            base = q * 4 * 2 * P
            dst = ot[:, base:base + np_q * 2 * P].rearrange("c (n two x) -> c n two x", two=2, x=P)
            src = pt[:, :np_q * P]
            nc.vector.tensor_copy(dst[:, :, 0, :], src[0:C, :].rearrange("c (n x) -> c n x", x=P))
            nc.scalar.copy(dst[:, :, 1, :], src[C:P, :].rearrange("c (n x) -> c n x", x=P))
        j = JBLK - 1
        pt = psumsp.tile([P, P], mybir.dt.float32, tag="pts")
        nc.tensor.transpose(pt[0:C, :], gt[:, j * C:(j + 1) * C], ident[:])
        nc.vector.tensor_copy(ot[:, j * P:(j + 1) * P], pt[0:C, :])
        nc.sync.dma_start(out[:, ci * XCHUNK:(ci + 1) * XCHUNK, :], ot[:].rearrange("c (x y) -> c x y", y=NY))

    CH_LO = NCHUNK_SPLIT

    # TC1: scatter low-j pillars (segments 0..3)
    with TileContext(nc) as tc:
        with (
            tc.tile_pool(name="scat", bufs=2) as scatp,
            tc.tile_pool(name="sidxp", bufs=2) as sidxp,
        ):
            for seg in range(NREG):
                emit_scatter(scatp, sidxp, seg)

    # TC2: scatter high-j pillars while streaming low-j chunks
    with TileContext(nc) as tc:
        with (
            tc.tile_pool(name="scat", bufs=2) as scatp,
            tc.tile_pool(name="sidxp", bufs=2) as sidxp,
            tc.tile_pool(name="const", bufs=1) as constp,
            tc.tile_pool(name="gather", bufs=4) as gatherp,
            tc.tile_pool(name="outp", bufs=3) as outp,
            tc.tile_pool(name="psum", bufs=4, space="PSUM") as psump,
            tc.tile_pool(name="psums", bufs=2, space="PSUM") as psumsp,
        ):
            for seg in range(NREG, 2 * NREG):
                emit_scatter(scatp, sidxp, seg)
            ident = constp.tile([P, P], mybir.dt.float32)
            make_identity(nc, ident[:])
            for ci in range(CH_LO):
                emit_chunk(ci, gatherp, outp, psump, psumsp, ident)

    # TC3: stream high-j chunks
    with TileContext(nc) as tc:
        with (
            tc.tile_pool(name="const", bufs=1) as constp,
            tc.tile_pool(name="gather", bufs=4) as gatherp,
            tc.tile_pool(name="outp", bufs=3) as outp,
            tc.tile_pool(name="psum", bufs=4, space="PSUM") as psump,
            tc.tile_pool(name="psums", bufs=2, space="PSUM") as psumsp,
        ):
            ident = constp.tile([P, P], mybir.dt.float32)
            make_identity(nc, ident[:])
            for ci in range(CH_LO, NCHUNK):
                emit_chunk(ci, gatherp, outp, psump, psumsp, ident)

    nc.finalize()
    return nc


def _prep_in_maps(feats_full, batch_indices, sample_indices):
    x = batch_indices[:, 2].astype(np.int64)
    y = batch_indices[:, 1].astype(np.int64)
    sm = sample_indices.astype(np.int64)
    xo = (NX - 1) - x
    h = xo // XH
    xl = xo % XH
    pos = xl * NY + y
    core = sm * 2 + h

    pp = pos % P            # partition
    jj = pos // P           # row within partition
    reg = pp // PREG
    half = (jj >= JSPLIT).astype(np.int64)
    seg = half * NREG + reg
    local = (pp % PREG) * RPP + jj   # int16-safe (< 26816)

    NSEG = 2 * NREG
    maxn = 0
    for k in range(NCORES):
        for g in range(NSEG):
            maxn = max(maxn, int(np.sum((core == k) & (seg == g))))
    jr = -(-(maxn + 1) // P) + 1     # ceil to 128 slots + 1 spare column

    nslot = P * jr
    in_maps = []
    for k in range(NCORES):
        feats_arr = np.zeros((NSEG * nslot, C), np.float32)
        idx_arr = np.full((16, NSEG * nslot // 16), 0, np.int16)
        for g in range(NSEG):
            rows = np.nonzero((core == k) & (seg == g))[0]
            loc = local[rows]
            order = np.argsort(loc)
            rows = rows[order]
            loc = loc[order]
            n = rows.size
            assert n <= nslot
            slots = np.arange(nslot)
            vals = np.full(nslot, 0, np.int16)
            vals[:n] = loc.astype(np.int16)
            vals[n:] = ((slots[n:] % P) % PREG) * RPP + JPP   # dump row, zero source
            # slot s lives at dram row (s % 128) * jr + s // 128 within the segment
            d = (slots[:n] % P) * jr + slots[:n] // P
            feats_arr[g * nslot + d] = feats_full[rows]
            idx_arr[:, g * (nslot // 16):(g + 1) * (nslot // 16)] = vals.reshape(nslot // 16, 16).T
        in_maps.append({"feats": feats_arr, "sidx": np.ascontiguousarray(np.tile(idx_arr, (8, 1)))})
    return in_maps, jr


def kernel(batch_pillar_features, batch_indices, sample_indices, batch_size):
    global LAST_RESULTS
    feats_full = np.asarray(batch_pillar_features, np.float32)
    batch_indices = np.asarray(batch_indices)
    sample_indices = np.asarray(sample_indices)
    bs = int(batch_size)
    assert bs == B and feats_full.shape[1] == C

    in_maps, jr = _prep_in_maps(feats_full, batch_indices, sample_indices)
    if _CACHE.get("jr") != jr:
        _CACHE["nc"] = _build_program(jr)
        _CACHE["jr"] = jr
    nc = _CACHE["nc"]

    res = run_bass_kernel_spmd(nc, in_maps, core_ids=list(range(NCORES)))
    LAST_RESULTS = res

    full = np.empty((B, C, NX, NY), np.float32)
    for k in range(NCORES):
        b, hh = k // 2, k % 2
        full[b, :, hh * XH:(hh + 1) * XH, :] = res.results[k]["out"]
    return full


# revision 8
# speedup vs baseline: 1.2773x; 1.2773x over previous
"""PointPillarScatter on 8 NeuronCores.

Full inputs -> full (B, C, NX, NY) float32 output.

Sharding: core k handles (sample b = k//2, output-x half h = k%2); each core
produces out[b, :, h*216:(h+1)*216, :] (the flip along x is baked into the
host-built scatter offsets).

Per-core device pipeline, two phases:

  Phase 1 (sparse scatter, ~6k rows/core):
    The ~6k real pillar rows are DMA'd densely into SBUF and scattered by
    dma_scatter_add into a runtime-pre-zeroed DRAM staging canvas.  Staging is
    laid out partition-major: partition p owns 838 consecutive rows (837 canvas
    positions {i : i % 128 == p} ordered by i // 128, plus 1 dump row for the
    padding slots), so the offsets bake in both the scatter and the
    transpose-friendly permutation, and int16 offsets stay in range per
    32-partition region.

  Phase 2 (dense stream, memory-bound):
    Per chunk of 24 output-x rows: one big contiguous DMA pulls the staging
    slice into SBUF as [128 pos-in-block, 93 blocks, 64 ch]; PE transposes
    pairs of 128-position blocks through an identity ([128,128] -> PSUM);
    DVE/ACT copy PSUM into the [64 ch, 11904 pos] out tile; one 3 MB DMA
    writes the (C, X, Y) canvas slice.
"""

import sys

sys.path.insert(0, "/opt/trn_rl_repo")

import numpy as np

import concourse.bacc as bacc
import concourse.mybir as mybir
from concourse.bass_utils import run_bass_kernel_spmd
from concourse.masks import make_identity
from concourse.tile import TileContext

C = 64
NX = 432
NY = 496
B = 4
NCORES = 8
XH = NX // 2            # 216 x-rows per core
M = XH * NY             # 107136 positions per core
P = 128
JPP = M // P            # 837 rows of 128 positions per partition
XCHUNK = 8
NCHUNK = XH // XCHUNK   # 27
MC = XCHUNK * NY        # 3968 positions per chunk
JBLK = MC // P          # 31 blocks of 128 positions
NCHUNK_SPLIT = 14       # chunks 0..13 read the low-j staging tensor
JSPLIT = NCHUNK_SPLIT * JBLK   # 434 low-j rows per partition
JLO = JSPLIT            # 434
JHI = JPP - JSPLIT      # 403
RPL = JLO + 1           # low staging rows per partition (+1 dump)
RPH = JHI + 1
NREG = 2                # int16 offsets cover 64 partitions x 435 rows
PREG = P // NREG        # 64 partitions per region

_CACHE = {}
LAST_RESULTS = None


def _build_program(jr):
    nslot = P * jr          # scatter slots per segment (padded, fixed count)
    NSEG = 2 * NREG         # (j-half, region)
    nc = bacc.Bacc(None, target_bir_lowering=False)
    feats = nc.dram_tensor("feats", [NSEG * nslot, C], mybir.dt.float32, kind="ExternalInput")
    sidx = nc.dram_tensor("sidx", [P, NSEG * nslot // 16], mybir.dt.int16, kind="ExternalInput")
    stlo = nc.dram_tensor("stlo", [P * RPL, C], mybir.dt.float32, kind="ExternalOutput")
    sthi = nc.dram_tensor("sthi", [P * RPH, C], mybir.dt.float32, kind="ExternalOutput")
    out = nc.dram_tensor("out", [C, XH, NY], mybir.dt.float32, kind="ExternalOutput")

    vlo = stlo[:].rearrange("(pt j) c -> pt j c", j=RPL)   # [128, 435, 64]
    vhi = sthi[:].rearrange("(pt j) c -> pt j c", j=RPH)   # [128, 404, 64]

    with TileContext(nc) as tc:
        with (
            tc.tile_pool(name="scat", bufs=2) as scatp,
            tc.tile_pool(name="sidxp", bufs=2) as sidxp,
            tc.tile_pool(name="const", bufs=1) as constp,
            tc.tile_pool(name="gather", bufs=4) as gatherp,
            tc.tile_pool(name="outp", bufs=3) as outp,
            tc.tile_pool(name="psum", bufs=4, space="PSUM") as psump,
            tc.tile_pool(name="psums", bufs=2, space="PSUM") as psumsp,
        ):
            for seg in range(NSEG):
                half, r = divmod(seg, NREG)
                target = stlo if half == 0 else sthi
                regrows = PREG * (RPL if half == 0 else RPH)
                ft = scatp.tile([P, jr, C], mybir.dt.float32, tag="ft")
                nc.scalar.dma_start(ft[:], feats[seg * nslot:(seg + 1) * nslot, :].rearrange("(p j) c -> p j c", j=jr))
                it = sidxp.tile([P, nslot // 16], mybir.dt.int16, tag="it")
                nc.scalar.dma_start(it[:], sidx[:, seg * (nslot // 16):(seg + 1) * (nslot // 16)])
                nc.gpsimd.dma_scatter_add(
                    out_ap=target[r * regrows:(r + 1) * regrows, :],
                    in_ap=ft[:],
                    idxs_ap=it[:],
                    num_idxs=nslot,
                    num_idxs_reg=nslot,
                    elem_size=C,
                    single_packet=False,
                )

            ident = constp.tile([P, P], mybir.dt.float32)
            make_identity(nc, ident[:])

            for ci in range(NCHUNK):
                if ci < NCHUNK_SPLIT:
                    src = vlo[:, ci * JBLK:(ci + 1) * JBLK, :]
                else:
                    src = vhi[:, (ci - NCHUNK_SPLIT) * JBLK:(ci - NCHUNK_SPLIT + 1) * JBLK, :]
                gt = gatherp.tile([P, JBLK * C], mybir.dt.float32, tag="gt")
                nc.scalar.dma_start(gt[:].rearrange("p (j c) -> p j c", c=C), src)

                ot = outp.tile([C, MC], mybir.dt.float32, tag="ot")
                npairs = JBLK // 2
                nquads = (npairs + 3) // 4
                for q in range(nquads):
                    np_q = min(4, npairs - q * 4)
                    pt = psump.tile([P, 512], mybir.dt.float32, tag="pt")
                    for m in range(np_q):
                        k = q * 4 + m
                        nc.tensor.transpose(pt[:, m * P:(m + 1) * P], gt[:, k * P:(k + 1) * P], ident[:])
                    base = q * 4 * 2 * P
                    dst = ot[:, base:base + np_q * 2 * P].rearrange("c (n two x) -> c n two x", two=2, x=P)
                    src_ps = pt[:, :np_q * P]
                    nc.vector.tensor_copy(dst[:, :, 0, :], src_ps[0:C, :].rearrange("c (n x) -> c n x", x=P))
                    nc.scalar.copy(dst[:, :, 1, :], src_ps[C:P, :].rearrange("c (n x) -> c n x", x=P))
                j = JBLK - 1
                pt = psumsp.tile([P, P], mybir.dt.float32, tag="pts")
                nc.tensor.transpose(pt[0:C, :], gt[:, j * C:(j + 1) * C], ident[:])
                nc.vector.tensor_copy(ot[:, j * P:(j + 1) * P], pt[0:C, :])
                nc.sync.dma_start(out[:, ci * XCHUNK:(ci + 1) * XCHUNK, :], ot[:].rearrange("c (x y) -> c x y", y=NY))

    nc.finalize()
    return nc


def _prep_in_maps(feats_full, batch_indices, sample_indices):
    x = batch_indices[:, 2].astype(np.int64)
    y = batch_indices[:, 1].astype(np.int64)
    sm = sample_indices.astype(np.int64)
    xo = (NX - 1) - x
    h = xo // XH
    xl = xo % XH
    pos = xl * NY + y
    core = sm * 2 + h

    pp = pos % P            # partition
    jj = pos // P           # row within partition
    reg = pp // PREG
    half = (jj >= JSPLIT).astype(np.int64)
    seg = half * NREG + reg
    rpx = np.where(half == 0, RPL, RPH)
    local = (pp % PREG) * rpx + np.where(half == 0, jj, jj - JSPLIT)  # int16-safe
    dumprow = np.where(half == 0, JLO, JHI)

    NSEG = 2 * NREG
    maxn = 0
    for k in range(NCORES):
        for g in range(NSEG):
            maxn = max(maxn, int(np.sum((core == k) & (seg == g))))
    jr = -(-(maxn + 1) // P) + 1     # ceil to 128 slots + 1 spare column

    nslot = P * jr
    in_maps = []
    for k in range(NCORES):
        feats_arr = np.zeros((NSEG * nslot, C), np.float32)
        idx_arr = np.full((16, NSEG * nslot // 16), 0, np.int16)
        for g in range(NSEG):
            half_g = g // NREG
            rp_g = RPL if half_g == 0 else RPH
            dump_g = (JLO if half_g == 0 else JHI)
            rows = np.nonzero((core == k) & (seg == g))[0]
            loc = local[rows]
            order = np.argsort(loc)
            rows = rows[order]
            loc = loc[order]
            n = rows.size
            assert n <= nslot
            slots = np.arange(nslot)
            vals = np.full(nslot, 0, np.int16)
            vals[:n] = loc.astype(np.int16)
            vals[n:] = ((slots[n:] % P) % PREG) * rp_g + dump_g   # dump row, zero source
            d = (slots[:n] % P) * jr + slots[:n] // P
            feats_arr[g * nslot + d] = feats_full[rows]
            idx_arr[:, g * (nslot // 16):(g + 1) * (nslot // 16)] = vals.reshape(nslot // 16, 16).T
        in_maps.append({"feats": feats_arr, "sidx": np.ascontiguousarray(np.tile(idx_arr, (8, 1)))})
    return in_maps, jr


def kernel(batch_pillar_features, batch_indices, sample_indices, batch_size):
    global LAST_RESULTS
    feats_full = np.asarray(batch_pillar_features, np.float32)
    batch_indices = np.asarray(batch_indices)
    sample_indices = np.asarray(sample_indices)
    bs = int(batch_size)
    assert bs == B and feats_full.shape[1] == C

    in_maps, jr = _prep_in_maps(feats_full, batch_indices, sample_indices)
    if _CACHE.get("jr") != jr:
        _CACHE["nc"] = _build_program(jr)
        _CACHE["jr"] = jr
    nc = _CACHE["nc"]

    res = run_bass_kernel_spmd(nc, in_maps, core_ids=list(range(NCORES)))
    LAST_RESULTS = res

    full = np.empty((B, C, NX, NY), np.float32)
    for k in range(NCORES):
        b, hh = k // 2, k % 2
        full[b, :, hh * XH:(hh + 1) * XH, :] = res.results[k]["out"]
    return full
